# revision 31
# baseline (speedup 1.0000x reference)
"""Causal self-attention (B=8, T=1024, C=1024, H=16, hd=64) on 8 TRN2 cores.

Sharding: data parallel - one batch element per NeuronCore.

v2: all matmul operands bf16 (PSUM accumulation stays fp32), host-prepped
slab-contiguous weight layouts (2KB+ DMA descriptors), every weight resident
in SBUF via upfront DMAs (no just-in-time slab gathers stalling the PE FIFO),
PE warmup matmuls during the initial DMA wait (HAM un-throttle), per-(hp,c)
inline softmax normalization, and output projection interleaved into the
last head-pair's attention as PE filler.

Device layouts (partition dim first):
  xsb      [128, 8*T]  x[b].T chunks; stationary for V proj, moving for Q/K.
  Q^T, K^T [128, 8*T]  head-pair hp in cols [hp*T,(hp+1)*T); per-head feature
           permutation (evens|odds per 32) folded into weights so RoPE's
           q1/q2 split is contiguous 16-row blocks.
  RoPE: qrot = (q + b)*C2 + (swap16(q) + swap16(b))*S2m via stream_shuffle.
  S^T  [s, t] per head: lhsT = Krot^T [64,128] (stationary), rhs = Qrot^T.
       Head pairs run concurrently in PE row groups 0-63 / 64-127.
  P~ = exp(S^T/8) on ACT out of PSUM (bf16); diagonal 128x128 gets a 0/1
       multiply on DVE.
  y^T  [65, t] = [V_j | ones].T @ P~ accumulated over s-tiles; row 64 is the
       softmax denominator r. 1/r broadcast via a K=2 matmul with e2.
  O^T  [e, t]  output projection of normalized Y^T; host transposes back.
"""
import numpy as np
import ml_dtypes
import concourse.bass as bass
import concourse.tile as tile
import concourse.mybir as mybir
from concourse import bacc
from concourse.bass_utils import run_bass_kernel_spmd

F32 = mybir.dt.float32
F32R = mybir.dt.float32r
BF16 = mybir.dt.bfloat16
EXP = mybir.ActivationFunctionType.Exp
IDENT = mybir.ActivationFunctionType.Identity
RECIP = mybir.ActivationFunctionType.Reciprocal
ADD = mybir.AluOpType.add
MULT = mybir.AluOpType.mult

B, T, C = 8, 1024, 1024
H, HD = 16, 64
NCORES = 8
TCH = T // 512


def build_program():
    nc = bacc.Bacc("TRN2", target_bir_lowering=False, debug=False)

    def din(name, shape, dt=BF16):
        return nc.dram_tensor(name, shape, dt, kind="ExternalInput").ap()

    xS = din("xS", [128, 8, T])            # [p, ct, t]
    wqS = din("wqS", [128, 8, 8, 128])     # [p, dblk, ct, m]
    wkS = din("wkS", [128, 8, 8, 128])
    wvS = din("wvS", [128, 2, 8, 512])     # [p, ch, ct, m]
    woS = din("woS", [128, 8, 8, 128])     # [p, eblk, dt, m]
    bq = din("bq", [128, 8], F32)
    bqs = din("bqs", [128, 8], F32)
    bk = din("bk", [128, 8], F32)
    bks = din("bks", [128, 8], F32)
    bo = din("bo", [128, 8], F32)
    bv = din("bv", [1, C])
    c2 = din("c2", [128, T], F32)
    s2m = din("s2m", [128, T], F32)
    tri = din("tri", [128, 128])
    onesrow = din("onesrow", [1, 128])
    ones16 = din("ones16", [128, 16])
    oT = nc.dram_tensor("oT", [C, T], F32, kind="ExternalOutput").ap()

    with tile.TileContext(nc) as tc:
        with (
            tc.tile_pool(name="pc", bufs=1) as pc,
            tc.tile_pool(name="prope", bufs=2) as prope,
            tc.tile_pool(name="ppt", bufs=4) as ppt,
            tc.tile_pool(name="pnorm", bufs=2) as pnorm,
            tc.tile_pool(name="posb", bufs=2) as posb,
            tc.tile_pool(name="psMM", bufs=2, space="PSUM") as psMM,
            tc.tile_pool(name="psY", bufs=2, space="PSUM") as psY,
            tc.tile_pool(name="psS", bufs=2, space="PSUM") as psS,
        ):
            S, Cq, G = nc.sync, nc.scalar, nc.gpsimd
            rr3 = [S, Cq, G]

            # ---- resident tensors; DMA issue order = priority ----
            wrm_sb = pc.tile([128, 256], BF16, tag="wrm")
            nc.vector.memset(wrm_sb[:], 0.0)
            xsb = pc.tile([128, 8 * T], BF16, tag="xbig")
            for ct in range(8):
                rr3[ct % 3].dma_start(xsb[:, ct * T:(ct + 1) * T], xS[:, ct, :])
            wq_sb = pc.tile([128, 8, 8, 128], BF16, tag="wq")
            wk_sb = pc.tile([128, 8, 8, 128], BF16, tag="wk")
            S.dma_start(wq_sb[:, 0], wqS[:, 0])
            Cq.dma_start(wk_sb[:, 0], wkS[:, 0])
            wv_sb = pc.tile([128, 2, 8, 512], BF16, tag="wv")
            G.dma_start(wv_sb[:, 0], wvS[:, 0])
            c2_sb = pc.tile([128, T], F32, tag="c2")
            s2_sb = pc.tile([128, T], F32, tag="s2")
            S.dma_start(c2_sb[:], c2)
            Cq.dma_start(s2_sb[:], s2m)
            tri_sb = pc.tile([128, 128], BF16, tag="tri")
            G.dma_start(tri_sb[:], tri)
            onesrow_sb = pc.tile([1, 128], BF16, tag="onesrow")
            S.dma_start(onesrow_sb[:], onesrow)
            bv_sb = pc.tile([1, C], BF16, tag="bv")
            G.dma_start(bv_sb[:], bv)
            btiles = {}
            for i, (nm, ap) in enumerate([("bq", bq), ("bqs", bqs), ("bk", bk),
                                          ("bks", bks), ("bo", bo)]):
                t_ = pc.tile([128, 8], F32, tag=nm)
                rr3[i % 3].dma_start(t_[:], ap)
                btiles[nm] = t_
            v_sb = [pc.tile([128, 16 * 65], BF16, tag=f"v{j}", name=f"v{j}")
                    for j in range(8)]
            v3 = [v_sb[j][:].rearrange("p (h j) -> p h j", j=65)
                  for j in range(8)]
            for j in range(8):
                rr3[j % 3].dma_start(v3[j][:, :, 64:65], ones16)
            for d in range(1, 8):
                rr3[d % 3].dma_start(wq_sb[:, d], wqS[:, d])
                rr3[(d + 1) % 3].dma_start(wk_sb[:, d], wkS[:, d])
            S.dma_start(wv_sb[:, 1], wvS[:, 1])
            wo_sb = pc.tile([128, 8, 8, 128], BF16, tag="wo")
            Cq.dma_start(wo_sb[:], woS)
            qrot_sb = pc.tile([128, 8 * T], BF16, tag="qrot")
            krot_sb = pc.tile([128, 8 * T], BF16, tag="krot")
            yt_sb = pc.tile([128, 8 * T], BF16, tag="yt")

            # ---- PE warmup: junk matmuls flip HAM to 8/8 and bridge the
            # initial x/weight DMA wait ----
            for w in range(90):
                wps = psMM.tile([128, 256], F32, tag="mm", name=f"wps{w}")
                nc.tensor.matmul(wps[:], wrm_sb[:, 0:128], wrm_sb[:],
                                 start=True, stop=True)

            # ---- emission helpers ----
            jcnt = [1000]

            def emit_junk(n):
                # HAM insurance: keep the PE busy through filler-less slots
                for _ in range(n):
                    jcnt[0] += 1
                    wps = psMM.tile([128, 256], F32, tag="mm",
                                    name=f"wj{jcnt[0]}")
                    nc.tensor.matmul(wps[:], wrm_sb[:, 0:128], wrm_sb[:],
                                     start=True, stop=True)

            def emit_qk_group(which, dblk, ch):
                wsb, bnm, bsnm, dest = which
                ps = psMM.tile([128, 512], F32, tag="mm",
                               name=f"p{bnm}{dblk}_{ch}")
                for ct in range(8):
                    nc.tensor.matmul(
                        ps[:], wsb[:, dblk, ct, :],
                        xsb[:, ct * T + ch * 512: ct * T + ch * 512 + 512],
                        start=(ct == 0), stop=(ct == 7))
                qsw = prope.tile([128, 512], F32, tag="qsw",
                                 name=f"qsw{bnm}{dblk}_{ch}")
                nc.vector.stream_shuffle(
                    qsw[:], ps[:],
                    mask=list(range(16, 32)) + list(range(0, 16)))
                qsw2 = prope.tile([128, 512], BF16, tag="qsw2",
                                  name=f"qs2{bnm}{dblk}_{ch}")
                dsl = dest[:, dblk * T + ch * 512: dblk * T + ch * 512 + 512]
                nc.vector.scalar_tensor_tensor(
                    dsl, ps[:], btiles[bnm][:, dblk:dblk + 1],
                    c2_sb[:, ch * 512:ch * 512 + 512], op0=ADD, op1=MULT)
                nc.vector.scalar_tensor_tensor(
                    qsw2[:], qsw[:], btiles[bsnm][:, dblk:dblk + 1],
                    s2_sb[:, ch * 512:ch * 512 + 512], op0=ADD, op1=MULT)
                nc.gpsimd.tensor_add(dsl, dsl, qsw2[:])

            def emit_v_group(ch, sblk):
                ps = psMM.tile([128, 512], F32, tag="mm", name=f"pv{ch}_{sblk}")
                for ct in range(8):
                    nc.tensor.matmul(
                        ps[:],
                        xsb[:, ct * T + sblk * 128: ct * T + sblk * 128 + 128],
                        wv_sb[:, ch, ct, :],
                        start=(ct == 0), stop=False)
                nc.tensor.matmul(
                    ps[:], onesrow_sb[:], bv_sb[:, ch * 512:(ch + 1) * 512],
                    start=False, stop=True)
                nc.vector.tensor_copy(v3[sblk][:, 8 * ch:8 * ch + 8, 0:64],
                                      ps[:])

            def emit_oproj_group(eblk, ch):
                ps = psMM.tile([128, 512], F32, tag="mm",
                               name=f"po{eblk}_{ch}")
                for dt in range(8):
                    nc.tensor.matmul(
                        ps[:], wo_sb[:, eblk, dt, :],
                        yt_sb[:, dt * T + ch * 512: dt * T + ch * 512 + 512],
                        start=(dt == 0), stop=(dt == 7))
                osb = posb.tile([128, 512], F32, tag="osb",
                                name=f"osb{eblk}_{ch}")
                if ch == 0:
                    nc.vector.tensor_scalar_add(osb[:], ps[:],
                                                btiles["bo"][:, eblk:eblk + 1])
                    rr3[eblk % 3].dma_start(
                        oT[eblk * 128:(eblk + 1) * 128,
                           ch * 512:(ch + 1) * 512],
                        osb[:])
                else:
                    # tail: halves so the store drain pipelines
                    for half in range(2):
                        sl = slice(half * 256, half * 256 + 256)
                        nc.vector.tensor_scalar_add(
                            osb[:, sl], ps[:, sl],
                            btiles["bo"][:, eblk:eblk + 1])
                        rr3[(2 * eblk + half) % 3].dma_start(
                            oT[eblk * 128:(eblk + 1) * 128,
                               ch * 512 + half * 256:
                               ch * 512 + half * 256 + 256],
                            osb[:, sl])

            QSPEC = (wq_sb, "bq", "bqs", qrot_sb)
            KSPEC = (wk_sb, "bk", "bks", krot_sb)

            # upfront: pair 0's Q/K strips + V chunk 0 (heads 0-7)
            for ch in range(TCH):
                emit_qk_group(QSPEC, 0, ch)
                emit_qk_group(KSPEC, 0, ch)
            for sblk in range(8):
                emit_v_group(0, sblk)

            # filler schedule: head-pair hp's fillers keep the PE dense
            # across the exp/mask latency of its attention inner loop
            fillers_by_hp = {hp: [] for hp in range(8)}
            for d in range(1, 8):
                for ch in range(TCH):
                    fillers_by_hp[d - 1].append(
                        lambda d=d, c=ch: emit_qk_group(QSPEC, d, c))
                    fillers_by_hp[d - 1].append(
                        lambda d=d, c=ch: emit_qk_group(KSPEC, d, c))
            for sblk in range(8):
                fillers_by_hp[sblk // 2].append(
                    lambda s=sblk: emit_v_group(1, s))

            # ---- attention with interleaved filler + inline normalization --
            pending_norm = []

            def emit_norm(hp, c):
                # broadcast r per head via two col-tiled K=1 matmuls
                # (concurrent), 1/r on ACT, then normalize yt in place
                rrs, ysl = pending_norm.pop(0)
                rbc = psMM.tile([128, 512], F32, tag="mm",
                                name=f"rbc{hp}_{c}")
                for hi in range(2):
                    nc.tensor.matmul(rbc[64 * hi:64 * hi + 64, :],
                                     onesrow_sb[:, 0:64], rrs[hi][:],
                                     start=True, stop=True)
                rinvb = pnorm.tile([128, 512], F32, tag="rinvb",
                                   name=f"rinvb{hp}_{c}")
                nc.vector.reciprocal_approx_fast(rinvb[:], rbc[:])
                nc.vector.tensor_mul(ysl, ysl, rinvb[:])

            LAG = 2
            for hp in range(8):
                fillers = fillers_by_hp[hp]
                base = hp * T
                # spread filler pops evenly over the 12 j-slots of this hp
                nslots = sum(4 * c + 4 + LAG for c in range(TCH))
                nf0 = [len(fillers)]
                npop = [0]
                slot = [0]

                def pop_fillers():
                    slot[0] += 1
                    popped = False
                    while fillers and npop[0] * nslots < slot[0] * nf0[0]:
                        fillers.pop(0)()
                        npop[0] += 1
                        popped = True
                    if not popped and hp >= 5:
                        emit_junk(3)

                for c in range(TCH):
                    if pending_norm:
                        # hp7/c1: norm(7,0) MUST precede the oproj fillers
                        # that read yt slab 7
                        pos = 0 if hp == 7 else min(1, len(fillers))
                        fillers.insert(pos,
                                       lambda h=hp, cc=c: emit_norm(h, cc))
                        nf0[0] += 1
                    njs = 4 * c + 4
                    ps_y = [psY.tile([65, 512], F32, tag="y",
                                     name=f"y{hp}_{c}_{k}")
                            for k in range(2)]
                    nsts = [128 * (j - 4 * c) if j >= 4 * c else 0
                            for j in range(njs)]
                    p_all = []
                    for j in range(njs):
                        nst = nsts[j]
                        ps_s = psS.tile([128, 1024], F32, tag="s")
                        ps_s3 = ps_s[:].rearrange("p (g t) -> p g t", g=2)
                        for hi in range(2):
                            r0 = 64 * hi
                            nc.tensor.matmul(
                                ps_s3[:, hi, nst:512],
                                krot_sb[r0:r0 + 64,
                                        base + j * 128: base + j * 128 + 128],
                                qrot_sb[r0:r0 + 64,
                                        base + c * 512 + nst: base + c * 512 + 512],
                                start=True, stop=True)
                        p_t = ppt.tile([128, 1024], BF16, tag="pt")
                        p_t3 = p_t[:].rearrange("p (g t) -> p g t", g=2)
                        nc.scalar.activation(p_t3[:, :, nst:512],
                                             ps_s3[:, :, nst:512],
                                             EXP, scale=0.125)
                        if j >= 4 * c:
                            for hi in range(2):
                                nc.vector.tensor_mul(
                                    p_t3[:, hi, nst:nst + 128],
                                    p_t3[:, hi, nst:nst + 128],
                                    tri_sb[:])
                        p_all.append(p_t3)
                        pop_fillers()
                        if j >= LAG:
                            jj = j - LAG
                            for hi in range(2):
                                h = 2 * hp + hi
                                nc.tensor.matmul(
                                    ps_y[hi][:, nsts[jj]:512],
                                    v_sb[jj][:, 65 * h: 65 * h + 65],
                                    p_all[jj][:, hi, nsts[jj]:512],
                                    start=(jj == 0), stop=False)
                    for jj in range(max(0, njs - LAG), njs):
                        pop_fillers()
                        for hi in range(2):
                            h = 2 * hp + hi
                            nc.tensor.matmul(
                                ps_y[hi][:, nsts[jj]:512],
                                v_sb[jj][:, 65 * h: 65 * h + 65],
                                p_all[jj][:, hi, nsts[jj]:512],
                                start=(jj == 0), stop=(jj == njs - 1))
                    # stash unnormalized y + reciprocals of the row sums;
                    # the rbc broadcast runs as next iteration's filler
                    rrs = [pnorm.tile([1, 512], BF16, tag=f"rr{k}",
                                      name=f"rr{hp}_{c}_{k}")
                           for k in range(2)]
                    for hi in range(2):
                        nc.scalar.activation(rrs[hi][:], ps_y[hi][64:65, :],
                                             IDENT)
                    ysl = yt_sb[:, base + c * 512: base + c * 512 + 512]
                    for hi in range(2):
                        nc.vector.tensor_copy(
                            yt_sb[64 * hi:64 * hi + 64,
                                  base + c * 512: base + c * 512 + 512],
                            ps_y[hi][0:64, :])
                    pending_norm.append((rrs, ysl))
                    # last head pair: output projection ch=0 becomes filler
                    # for c=1 (norm(7,0) is queued ahead of it)
                    if hp == 7 and c == 0:
                        for eblk in range(8):
                            fillers.append(
                                lambda e=eblk: emit_oproj_group(e, 0))
                        nf0[0] = npop[0] + len(fillers)

                # flush remaining fillers at end of the head pair
                for f in fillers:
                    f()
                fillers_by_hp[hp] = []

            # ---- tail: last norm + output projection ch=1 ----
            emit_norm(7, 1)
            for eblk in range(8):
                emit_oproj_group(eblk, 1)

    nc.compile()
    return nc


def prep_inputs(x, wq, bq, wk, bk, wv, bv, wo, bo):
    """Host-side prep: per-head feature permutation, slab layouts, RoPE
    tables, bf16 casts."""
    f32 = np.float32
    bf16 = ml_dtypes.bfloat16
    # interleave-16 feature order per head: [q1[0:16], q2[0:16],
    # q1[16:32], q2[16:32]] where q1 = even orig features, q2 = odd.
    perm = np.concatenate([
        np.arange(0, 32, 2), np.arange(1, 32, 2),
        np.arange(32, 64, 2), np.arange(33, 64, 2),
    ])
    pidx = np.concatenate([h * HD + perm for h in range(H)])

    wq_p, bq_p = wq[pidx], bq[pidx]
    wk_p, bk_p = wk[pidx], bk[pidx]
    # swap the 16-row halves within every 32-row quadrant
    swap = lambda v: np.ascontiguousarray(
        v.reshape(2 * H, 2, 16)[:, ::-1].reshape(-1))
    bt = lambda v: np.ascontiguousarray(v.reshape(8, 128).T, dtype=f32)

    # slab layouts: [p, blk, ct, m] so each slab DMA is contiguous per
    # partition (2KB descriptors instead of 512B gathers)
    def qk_slab(w):  # w: [d_out, c_in] torch Linear weight (permuted)
        wT = w.T  # [c_in, d_out]
        return np.ascontiguousarray(
            wT.reshape(8, 128, 8, 128).transpose(1, 2, 0, 3), dtype=bf16)

    def v_slab(w):
        wT = w.T
        return np.ascontiguousarray(
            wT.reshape(8, 128, 2, 512).transpose(1, 2, 0, 3), dtype=bf16)

    inv_freq = (1.0 / (10000.0 ** (np.arange(0, HD, 2, dtype=np.float64) / HD)))
    th = np.outer(np.arange(T, dtype=np.float64), inv_freq)  # [T, 32]
    cosT = np.cos(th).T.astype(f32)  # [32, T]
    sinT = np.sin(th).T.astype(f32)
    c64 = np.concatenate([cosT[0:16], cosT[0:16], cosT[16:32], cosT[16:32]])
    s64 = np.concatenate([-sinT[0:16], sinT[0:16], -sinT[16:32], sinT[16:32]])
    c2 = np.ascontiguousarray(np.tile(c64, (2, 1)))  # [128, T]
    s2m = np.ascontiguousarray(np.tile(s64, (2, 1)))

    shared = {
        "wqS": qk_slab(wq_p),
        "wkS": qk_slab(wk_p),
        "wvS": v_slab(wv),
        "woS": qk_slab(wo),
        "bq": bt(bq_p), "bqs": bt(swap(bq_p)),
        "bk": bt(bk_p), "bks": bt(swap(bk_p)),
        "bo": bt(bo),
        "bv": np.ascontiguousarray(bv[None, :], dtype=bf16),
        "c2": c2, "s2m": s2m,
        "tri": np.triu(np.ones((128, 128), dtype=bf16)),
        "onesrow": np.ones((1, 128), dtype=bf16),
        "ones16": np.ones((128, 16), dtype=bf16),
    }
    in_maps = []
    for b in range(B):
        m = dict(shared)
        xT = np.asarray(x[b]).T  # [C, T]
        m["xS"] = np.ascontiguousarray(
            xT.reshape(8, 128, T).transpose(1, 0, 2), dtype=bf16)
        in_maps.append(m)
    return in_maps


_nc_cache = None


def run(inputs, trace=False, trace_kwargs=None):
    global _nc_cache
    if _nc_cache is None:
        _nc_cache = build_program()
    in_maps = prep_inputs(
        np.asarray(inputs["x"], dtype=np.float32),
        *[np.asarray(inputs[k], dtype=np.float32)
          for k in ["wq", "bq", "wk", "bk", "wv", "bv", "wo", "bo"]])
    res = run_bass_kernel_spmd(_nc_cache, in_maps, list(range(NCORES)),
                               trace=trace, **(trace_kwargs or {}))
    out = np.stack([np.ascontiguousarray(res.results[b]["oT"].T)
                    for b in range(B)]).astype(np.float32)
    return out, res


def kernel(**inputs):
    out, _ = run(inputs, trace=False)
    return out


# revision 32
# speedup vs baseline: 1.0414x; 1.0414x over previous
"""Causal self-attention (B=8, T=1024, C=1024, H=16, hd=64) on 8 TRN2 cores.

Sharding: data parallel - one batch element per NeuronCore.

v2: all matmul operands bf16 (PSUM accumulation stays fp32), host-prepped
slab-contiguous weight layouts (2KB+ DMA descriptors), every weight resident
in SBUF via upfront DMAs (no just-in-time slab gathers stalling the PE FIFO),
PE warmup matmuls during the initial DMA wait (HAM un-throttle), per-(hp,c)
inline softmax normalization, and output projection interleaved into the
last head-pair's attention as PE filler.

Device layouts (partition dim first):
  xsb      [128, 8*T]  x[b].T chunks; stationary for V proj, moving for Q/K.
  Q^T, K^T [128, 8*T]  head-pair hp in cols [hp*T,(hp+1)*T); per-head feature
           permutation (evens|odds per 32) folded into weights so RoPE's
           q1/q2 split is contiguous 16-row blocks.
  RoPE: qrot = (q + b)*C2 + (swap16(q) + swap16(b))*S2m via stream_shuffle.
  S^T  [s, t] per head: lhsT = Krot^T [64,128] (stationary), rhs = Qrot^T.
       Head pairs run concurrently in PE row groups 0-63 / 64-127.
  P~ = exp(S^T/8) on ACT out of PSUM (bf16); diagonal 128x128 gets a 0/1
       multiply on DVE.
  y^T  [65, t] = [V_j | ones].T @ P~ accumulated over s-tiles; row 64 is the
       softmax denominator r. 1/r broadcast via a K=2 matmul with e2.
  O^T  [e, t]  output projection of normalized Y^T; host transposes back.
"""
import numpy as np
import ml_dtypes
import concourse.bass as bass
import concourse.tile as tile
import concourse.mybir as mybir
from concourse import bacc
from concourse.bass_utils import run_bass_kernel_spmd

F32 = mybir.dt.float32
F32R = mybir.dt.float32r
BF16 = mybir.dt.bfloat16
EXP = mybir.ActivationFunctionType.Exp
IDENT = mybir.ActivationFunctionType.Identity
RECIP = mybir.ActivationFunctionType.Reciprocal
ADD = mybir.AluOpType.add
MULT = mybir.AluOpType.mult

B, T, C = 8, 1024, 1024
H, HD = 16, 64
NCORES = 8
TCH = T // 512


def build_program():
    nc = bacc.Bacc("TRN2", target_bir_lowering=False, debug=False)

    def din(name, shape, dt=BF16):
        return nc.dram_tensor(name, shape, dt, kind="ExternalInput").ap()

    xS = din("xS", [128, 8, T])            # [p, ct, t]
    wqS = din("wqS", [128, 8, 8, 128])     # [p, dblk, ct, m]
    wkS = din("wkS", [128, 8, 8, 128])
    wvS = din("wvS", [128, 2, 8, 512])     # [p, ch, ct, m]
    woS = din("woS", [128, 8, 8, 128])     # [p, eblk, dt, m]
    bq = din("bq", [128, 8], F32)
    bqs = din("bqs", [128, 8], F32)
    bk = din("bk", [128, 8], F32)
    bks = din("bks", [128, 8], F32)
    bo = din("bo", [128, 8], F32)
    bv = din("bv", [1, C])
    c2 = din("c2", [128, T], F32)
    s2m = din("s2m", [128, T], F32)
    tri = din("tri", [128, 128])
    onesrow = din("onesrow", [1, 128])
    ones16 = din("ones16", [128, 16])
    oT = nc.dram_tensor("oT", [C, T], F32, kind="ExternalOutput").ap()

    with tile.TileContext(nc) as tc:
        with (
            tc.tile_pool(name="pc", bufs=1) as pc,
            tc.tile_pool(name="prope", bufs=2) as prope,
            tc.tile_pool(name="ppt", bufs=4) as ppt,
            tc.tile_pool(name="pnorm", bufs=2) as pnorm,
            tc.tile_pool(name="posb", bufs=2) as posb,
            tc.tile_pool(name="psMM", bufs=2, space="PSUM") as psMM,
            tc.tile_pool(name="psY", bufs=2, space="PSUM") as psY,
            tc.tile_pool(name="psS", bufs=2, space="PSUM") as psS,
        ):
            S, Cq, G = nc.sync, nc.scalar, nc.gpsimd
            rr3 = [S, Cq, G]

            # ---- resident tensors; DMA issue order = priority ----
            wrm_sb = pc.tile([128, 256], BF16, tag="wrm")
            nc.gpsimd.memset(wrm_sb[:], 0.0)
            xsb = pc.tile([128, 8 * T], BF16, tag="xbig")
            for ct in range(8):
                rr3[ct % 3].dma_start(xsb[:, ct * T:(ct + 1) * T], xS[:, ct, :])
            wq_sb = pc.tile([128, 8, 8, 128], BF16, tag="wq")
            wk_sb = pc.tile([128, 8, 8, 128], BF16, tag="wk")
            S.dma_start(wq_sb[:, 0], wqS[:, 0])
            Cq.dma_start(wk_sb[:, 0], wkS[:, 0])
            wv_sb = pc.tile([128, 2, 8, 512], BF16, tag="wv")
            G.dma_start(wv_sb[:, 0], wvS[:, 0])
            c2_sb = pc.tile([128, T], F32, tag="c2")
            s2_sb = pc.tile([128, T], F32, tag="s2")
            S.dma_start(c2_sb[:], c2)
            Cq.dma_start(s2_sb[:], s2m)
            tri_sb = pc.tile([128, 128], BF16, tag="tri")
            G.dma_start(tri_sb[:], tri)
            onesrow_sb = pc.tile([1, 128], BF16, tag="onesrow")
            S.dma_start(onesrow_sb[:], onesrow)
            bv_sb = pc.tile([1, C], BF16, tag="bv")
            G.dma_start(bv_sb[:], bv)
            btiles = {}
            for i, (nm, ap) in enumerate([("bq", bq), ("bqs", bqs), ("bk", bk),
                                          ("bks", bks), ("bo", bo)]):
                t_ = pc.tile([128, 8], F32, tag=nm)
                rr3[i % 3].dma_start(t_[:], ap)
                btiles[nm] = t_
            v_sb = [pc.tile([128, 16 * 65], BF16, tag=f"v{j}", name=f"v{j}")
                    for j in range(8)]
            v3 = [v_sb[j][:].rearrange("p (h j) -> p h j", j=65)
                  for j in range(8)]
            for j in range(8):
                rr3[j % 3].dma_start(v3[j][:, :, 64:65], ones16)
            for d in range(1, 8):
                rr3[d % 3].dma_start(wq_sb[:, d], wqS[:, d])
                rr3[(d + 1) % 3].dma_start(wk_sb[:, d], wkS[:, d])
            S.dma_start(wv_sb[:, 1], wvS[:, 1])
            wo_sb = pc.tile([128, 8, 8, 128], BF16, tag="wo")
            Cq.dma_start(wo_sb[:], woS)
            qrot_sb = pc.tile([128, 8 * T], BF16, tag="qrot")
            krot_sb = pc.tile([128, 8 * T], BF16, tag="krot")
            yt_sb = pc.tile([128, 8 * T], BF16, tag="yt")

            # ---- PE warmup: junk matmuls flip HAM to 8/8 and bridge the
            # initial x/weight DMA wait ----
            for w in range(90):
                wps = psMM.tile([128, 256], F32, tag="mm", name=f"wps{w}")
                nc.tensor.matmul(wps[:], wrm_sb[:, 0:128], wrm_sb[:],
                                 start=True, stop=True)

            # ---- emission helpers ----
            def emit_qk_group(which, dblk, ch):
                wsb, bnm, bsnm, dest = which
                ps = psMM.tile([128, 512], F32, tag="mm",
                               name=f"p{bnm}{dblk}_{ch}")
                for ct in range(8):
                    nc.tensor.matmul(
                        ps[:], wsb[:, dblk, ct, :],
                        xsb[:, ct * T + ch * 512: ct * T + ch * 512 + 512],
                        start=(ct == 0), stop=(ct == 7))
                qsw = prope.tile([128, 512], F32, tag="qsw",
                                 name=f"qsw{bnm}{dblk}_{ch}")
                nc.vector.stream_shuffle(
                    qsw[:], ps[:],
                    mask=list(range(16, 32)) + list(range(0, 16)))
                qsw2 = prope.tile([128, 512], BF16, tag="qsw2",
                                  name=f"qs2{bnm}{dblk}_{ch}")
                dsl = dest[:, dblk * T + ch * 512: dblk * T + ch * 512 + 512]
                nc.vector.scalar_tensor_tensor(
                    dsl, ps[:], btiles[bnm][:, dblk:dblk + 1],
                    c2_sb[:, ch * 512:ch * 512 + 512], op0=ADD, op1=MULT)
                nc.vector.scalar_tensor_tensor(
                    qsw2[:], qsw[:], btiles[bsnm][:, dblk:dblk + 1],
                    s2_sb[:, ch * 512:ch * 512 + 512], op0=ADD, op1=MULT)
                nc.gpsimd.tensor_add(dsl, dsl, qsw2[:])

            def emit_v_group(ch, sblk):
                ps = psMM.tile([128, 512], F32, tag="mm", name=f"pv{ch}_{sblk}")
                for ct in range(8):
                    nc.tensor.matmul(
                        ps[:],
                        xsb[:, ct * T + sblk * 128: ct * T + sblk * 128 + 128],
                        wv_sb[:, ch, ct, :],
                        start=(ct == 0), stop=False)
                nc.tensor.matmul(
                    ps[:], onesrow_sb[:], bv_sb[:, ch * 512:(ch + 1) * 512],
                    start=False, stop=True)
                nc.vector.tensor_copy(v3[sblk][:, 8 * ch:8 * ch + 8, 0:64],
                                      ps[:])

            def emit_oproj_group(eblk, ch):
                ps = psMM.tile([128, 512], F32, tag="mm",
                               name=f"po{eblk}_{ch}")
                for dt in range(8):
                    nc.tensor.matmul(
                        ps[:], wo_sb[:, eblk, dt, :],
                        yt_sb[:, dt * T + ch * 512: dt * T + ch * 512 + 512],
                        start=(dt == 0), stop=(dt == 7))
                osb = posb.tile([128, 512], F32, tag="osb",
                                name=f"osb{eblk}_{ch}")
                nc.vector.tensor_scalar_add(osb[:], ps[:],
                                            btiles["bo"][:, eblk:eblk + 1])
                rr3[eblk % 3].dma_start(
                    oT[eblk * 128:(eblk + 1) * 128, ch * 512:(ch + 1) * 512],
                    osb[:])

            QSPEC = (wq_sb, "bq", "bqs", qrot_sb)
            KSPEC = (wk_sb, "bk", "bks", krot_sb)

            # upfront: pair 0's Q/K strips + V chunk 0 (heads 0-7)
            for ch in range(TCH):
                emit_qk_group(QSPEC, 0, ch)
                emit_qk_group(KSPEC, 0, ch)
            for sblk in range(8):
                emit_v_group(0, sblk)

            # filler schedule: head-pair hp's fillers keep the PE dense
            # across the exp/mask latency of its attention inner loop
            fillers_by_hp = {hp: [] for hp in range(8)}
            for d in range(1, 8):
                for ch in range(TCH):
                    fillers_by_hp[d - 1].append(
                        lambda d=d, c=ch: emit_qk_group(QSPEC, d, c))
                    fillers_by_hp[d - 1].append(
                        lambda d=d, c=ch: emit_qk_group(KSPEC, d, c))
            for sblk in range(8):
                fillers_by_hp[sblk // 2].append(
                    lambda s=sblk: emit_v_group(1, s))

            # ---- attention with interleaved filler + inline normalization --
            pending_norm = []

            def emit_norm(hp, c):
                # broadcast r per head via two col-tiled K=1 matmuls
                # (concurrent), 1/r on ACT, then normalize yt in place
                rrs, ysl = pending_norm.pop(0)
                rbc = psMM.tile([128, 512], F32, tag="mm",
                                name=f"rbc{hp}_{c}")
                for hi in range(2):
                    nc.tensor.matmul(rbc[64 * hi:64 * hi + 64, :],
                                     onesrow_sb[:, 0:64], rrs[hi][:],
                                     start=True, stop=True)
                rinvb = pnorm.tile([128, 512], F32, tag="rinvb",
                                   name=f"rinvb{hp}_{c}")
                nc.vector.reciprocal_approx_fast(rinvb[:], rbc[:])
                nc.vector.tensor_mul(ysl, ysl, rinvb[:])

            LAG = 2
            for hp in range(8):
                fillers = fillers_by_hp[hp]
                base = hp * T
                # spread filler pops evenly over the 12 j-slots of this hp
                nslots = sum(4 * c + 4 + LAG for c in range(TCH))
                nf0 = [len(fillers)]
                npop = [0]
                slot = [0]

                def pop_fillers():
                    slot[0] += 1
                    while fillers and npop[0] * nslots < slot[0] * nf0[0]:
                        fillers.pop(0)()
                        npop[0] += 1

                for c in range(TCH):
                    if pending_norm:
                        # hp7/c1: norm(7,0) MUST precede the oproj fillers
                        # that read yt slab 7
                        pos = 0 if hp == 7 else min(1, len(fillers))
                        fillers.insert(pos,
                                       lambda h=hp, cc=c: emit_norm(h, cc))
                        nf0[0] += 1
                    njs = 4 * c + 4
                    ps_y = [psY.tile([65, 512], F32, tag="y",
                                     name=f"y{hp}_{c}_{k}")
                            for k in range(2)]
                    nsts = [128 * (j - 4 * c) if j >= 4 * c else 0
                            for j in range(njs)]
                    p_all = []
                    for j in range(njs):
                        nst = nsts[j]
                        ps_s = psS.tile([128, 1024], F32, tag="s")
                        ps_s3 = ps_s[:].rearrange("p (g t) -> p g t", g=2)
                        for hi in range(2):
                            r0 = 64 * hi
                            nc.tensor.matmul(
                                ps_s3[:, hi, nst:512],
                                krot_sb[r0:r0 + 64,
                                        base + j * 128: base + j * 128 + 128],
                                qrot_sb[r0:r0 + 64,
                                        base + c * 512 + nst: base + c * 512 + 512],
                                start=True, stop=True)
                        p_t = ppt.tile([128, 1024], BF16, tag="pt")
                        p_t3 = p_t[:].rearrange("p (g t) -> p g t", g=2)
                        nc.scalar.activation(p_t3[:, :, nst:512],
                                             ps_s3[:, :, nst:512],
                                             EXP, scale=0.125)
                        if j >= 4 * c:
                            for hi in range(2):
                                nc.vector.tensor_mul(
                                    p_t3[:, hi, nst:nst + 128],
                                    p_t3[:, hi, nst:nst + 128],
                                    tri_sb[:])
                        p_all.append(p_t3)
                        pop_fillers()
                        if j >= LAG:
                            jj = j - LAG
                            for hi in range(2):
                                h = 2 * hp + hi
                                nc.tensor.matmul(
                                    ps_y[hi][:, nsts[jj]:512],
                                    v_sb[jj][:, 65 * h: 65 * h + 65],
                                    p_all[jj][:, hi, nsts[jj]:512],
                                    start=(jj == 0), stop=False)
                    for jj in range(max(0, njs - LAG), njs):
                        pop_fillers()
                        for hi in range(2):
                            h = 2 * hp + hi
                            nc.tensor.matmul(
                                ps_y[hi][:, nsts[jj]:512],
                                v_sb[jj][:, 65 * h: 65 * h + 65],
                                p_all[jj][:, hi, nsts[jj]:512],
                                start=(jj == 0), stop=(jj == njs - 1))
                    # stash unnormalized y + reciprocals of the row sums;
                    # the rbc broadcast runs as next iteration's filler
                    rrs = [pnorm.tile([1, 512], BF16, tag=f"rr{k}",
                                      name=f"rr{hp}_{c}_{k}")
                           for k in range(2)]
                    for hi in range(2):
                        nc.scalar.activation(rrs[hi][:], ps_y[hi][64:65, :],
                                             IDENT)
                    ysl = yt_sb[:, base + c * 512: base + c * 512 + 512]
                    for hi in range(2):
                        nc.vector.tensor_copy(
                            yt_sb[64 * hi:64 * hi + 64,
                                  base + c * 512: base + c * 512 + 512],
                            ps_y[hi][0:64, :])
                    pending_norm.append((rrs, ysl))
                    # last head pair: output projection ch=0 becomes filler
                    # for c=1 (norm(7,0) is queued ahead of it)
                    if hp == 7 and c == 0:
                        for eblk in range(8):
                            fillers.append(
                                lambda e=eblk: emit_oproj_group(e, 0))
                        nf0[0] = npop[0] + len(fillers)

                # flush remaining fillers at end of the head pair
                for f in fillers:
                    f()
                fillers_by_hp[hp] = []

            # ---- tail: last norm + output projection ch=1 ----
            emit_norm(7, 1)
            for eblk in range(8):
                emit_oproj_group(eblk, 1)

    nc.compile()
    return nc


def prep_inputs(x, wq, bq, wk, bk, wv, bv, wo, bo):
    """Host-side prep: per-head feature permutation, slab layouts, RoPE
    tables, bf16 casts."""
    f32 = np.float32
    bf16 = ml_dtypes.bfloat16
    # interleave-16 feature order per head: [q1[0:16], q2[0:16],
    # q1[16:32], q2[16:32]] where q1 = even orig features, q2 = odd.
    perm = np.concatenate([
        np.arange(0, 32, 2), np.arange(1, 32, 2),
        np.arange(32, 64, 2), np.arange(33, 64, 2),
    ])
    pidx = np.concatenate([h * HD + perm for h in range(H)])

    wq_p, bq_p = wq[pidx], bq[pidx]
    wk_p, bk_p = wk[pidx], bk[pidx]
    # swap the 16-row halves within every 32-row quadrant
    swap = lambda v: np.ascontiguousarray(
        v.reshape(2 * H, 2, 16)[:, ::-1].reshape(-1))
    bt = lambda v: np.ascontiguousarray(v.reshape(8, 128).T, dtype=f32)

    # slab layouts: [p, blk, ct, m] so each slab DMA is contiguous per
    # partition (2KB descriptors instead of 512B gathers)
    def qk_slab(w):  # w: [d_out, c_in] torch Linear weight (permuted)
        wT = w.T  # [c_in, d_out]
        return np.ascontiguousarray(
            wT.reshape(8, 128, 8, 128).transpose(1, 2, 0, 3), dtype=bf16)

    def v_slab(w):
        wT = w.T
        return np.ascontiguousarray(
            wT.reshape(8, 128, 2, 512).transpose(1, 2, 0, 3), dtype=bf16)

    inv_freq = (1.0 / (10000.0 ** (np.arange(0, HD, 2, dtype=np.float64) / HD)))
    th = np.outer(np.arange(T, dtype=np.float64), inv_freq)  # [T, 32]
    cosT = np.cos(th).T.astype(f32)  # [32, T]
    sinT = np.sin(th).T.astype(f32)
    c64 = np.concatenate([cosT[0:16], cosT[0:16], cosT[16:32], cosT[16:32]])
    s64 = np.concatenate([-sinT[0:16], sinT[0:16], -sinT[16:32], sinT[16:32]])
    c2 = np.ascontiguousarray(np.tile(c64, (2, 1)))  # [128, T]
    s2m = np.ascontiguousarray(np.tile(s64, (2, 1)))

    shared = {
        "wqS": qk_slab(wq_p),
        "wkS": qk_slab(wk_p),
        "wvS": v_slab(wv),
        "woS": qk_slab(wo),
        "bq": bt(bq_p), "bqs": bt(swap(bq_p)),
        "bk": bt(bk_p), "bks": bt(swap(bk_p)),
        "bo": bt(bo),
        "bv": np.ascontiguousarray(bv[None, :], dtype=bf16),
        "c2": c2, "s2m": s2m,
        "tri": np.triu(np.ones((128, 128), dtype=bf16)),
        "onesrow": np.ones((1, 128), dtype=bf16),
        "ones16": np.ones((128, 16), dtype=bf16),
    }
    in_maps = []
    for b in range(B):
        m = dict(shared)
        xT = np.asarray(x[b]).T  # [C, T]
        m["xS"] = np.ascontiguousarray(
            xT.reshape(8, 128, T).transpose(1, 0, 2), dtype=bf16)
        in_maps.append(m)
    return in_maps


_nc_cache = None


def run(inputs, trace=False, trace_kwargs=None):
    global _nc_cache
    if _nc_cache is None:
        _nc_cache = build_program()
    in_maps = prep_inputs(
        np.asarray(inputs["x"], dtype=np.float32),
        *[np.asarray(inputs[k], dtype=np.float32)
          for k in ["wq", "bq", "wk", "bk", "wv", "bv", "wo", "bo"]])
    res = run_bass_kernel_spmd(_nc_cache, in_maps, list(range(NCORES)),
                               trace=trace, **(trace_kwargs or {}))
    out = np.stack([np.ascontiguousarray(res.results[b]["oT"].T)
                    for b in range(B)]).astype(np.float32)
    return out, res


def kernel(**inputs):
    out, _ = run(inputs, trace=False)
    return out
